# revision 31
# baseline (speedup 1.0000x reference)
"""DyRep intensity/survival kernel for 8 Trainium2 NeuronCores.

Strategy
--------
The reference computes, per event (u, v) and for both event types k:
    g_k(a, b) = 0.5*(cat(za, zb) @ W_k.T + cat(zb, za) @ W_k.T) + b_k
              = (za + zb) @ Wc_k + b_k ,   Wc = 0.5*(W[:, :H] + W[:, H:]).T
so every lambda only depends on the 2-vector projection P[n] = emb[n] @ Wc.
We therefore:
  1. shard the embedding table row-wise across the 8 cores; each core
     computes its P-shard with one small PE matmul (a ones-feature row
     folds b/2 into the projection),
  2. AllGather the [50000, 2] projection table (400KB) into DRAM,
  3. spread the table into 256B-stride fat rows (4 nodes each) and gather
     the 42 rows each event needs (u, v, 20+20 negative samples) with
     SWDGE dma_gather (32B elements, int16 fat indices, <=896 idx per
     instruction -- the Q7 idx scratch caps num_idxs around 1K), then
     pick the wanted node lane with two DVE selects,
  4. compute softplus (Exp then Ln(1+x)) on ACT and reduce on DVE.
Events are data-parallel: core c owns events [2048c, 2048(c+1)).
"""

import numpy as np
from contextlib import ExitStack

import concourse.bass as bass
import concourse.bacc as bacc
import concourse.tile as tile
from concourse import mybir
from concourse.bass_utils import run_bass_kernel_spmd

F32 = mybir.dt.float32
I32 = mybir.dt.int32

N_NODES = 50000
H = 32
B = 16384
S = 20
NCORES = 8
P = 128

BC = B // NCORES            # events per core (2048)
EV_P = BC // P              # events per partition (16)
SLOT = 2 + 2 * S            # gathered rows per event (42)
FREE_I = EV_P * SLOT        # gathered rows per partition (672)
RSH = N_NODES // NCORES     # embedding rows per shard (6250)
TSH = (RSH + P - 1) // P    # matmul tiles per shard (49)
PADN = P * TSH              # padded rows per shard (6272)
CH_EV = 2                   # events per partition per gather chunk
NCH = EV_P // CH_EV         # chunks (8)
CH_I = CH_EV * SLOT         # rows gathered per partition per chunk (84)
GW = 2 * SLOT               # gathered f32 per event (84)
EW = 2 + 4 * S              # g/f values per event (82)
PACK = 4                    # table rows packed per 256B fat row
FATE = 2 * PACK             # f32 elements gathered per lookup (8)
NFAT = NCORES * PADN // PACK  # fat rows (12544)
SUB = 12                    # sub-gathers per chunk
SLOT_SUB = CH_I // SUB      # slots per sub-gather (7)
NI_SUB = SLOT_SUB * P       # num_idxs per sub-gather (896)
IWS = NI_SUB // 16          # wrapped idx cols per sub-gather (56)
_SEMCNT = 0

_PROG = None
last_results = None         # BassKernelResults of the most recent run
I16 = mybir.dt.int16
U8 = mybir.dt.uint8


def _emit_dma_gather(nc, out_ap, in_ap, idxs_ap, num_idxs, elem_size, elem_step):
    """nc.gpsimd.dma_gather without the 256B elem_size guard.

    The Q7 descriptor generator supports any elem_size (packets loop); only
    the DRAM row stride is encoded in 256B units. Mirrors bass.py dma_gather
    for the non-transpose, DRAM-source, immediate-trigger case.
    """
    from concourse import ap_utils
    from concourse._compat import exact_div, round_up_to_multiple

    gp = nc.gpsimd
    assert idxs_ap.dtype == I16
    assert in_ap.dtype == out_ap.dtype
    assert num_idxs % 128 == 0
    assert in_ap.ap[-1][1] == out_ap.ap[-1][1] == elem_size
    assert out_ap.ap[0][1] * out_ap.ap[1][1] == round_up_to_multiple(num_idxs, 128)
    assert in_ap.ap[0][0] == elem_step
    stride_bytes = elem_step * mybir.dt.size(in_ap.dtype)
    stride_bytes_256 = exact_div(stride_bytes, 256)
    assert 0 < stride_bytes_256 < 256
    assert ap_utils.ap_is_contiguous(out_ap.ap[1:])
    assert ap_utils.ap_is_contiguous(idxs_ap.ap[1:])

    _in_ap = gp.lower_ap_dma(in_ap, for_custom_bir_dma=True)
    _idxs_ap = gp.lower_ap(idxs_ap)
    _out_ap = gp.lower_ap(out_ap)
    return gp.add_instruction(
        mybir.InstDMAGatherAnt(
            name=nc.get_next_instruction_name(),
            ins=[
                *_in_ap,
                _idxs_ap,
                gp.lower_val_access(gp.to_reg(num_idxs)),
            ],
            outs=[_out_ap],
            transpose=False,
            num_idxs=num_idxs,
            elem_size=elem_size,
            stride_bytes_256=stride_bytes_256,
            gen_mode=0,
            single_packet=True,
            queue_num=0,
            sbuf_tokens_per_rank=0,
            sbuf_free_dim_per_rank=0,
            sbuf_free_dim_pad_per_rank=0,
            sbuf_byte_offset=0,
        )
    )


def build_program(reps=1, debug=False):
    nc = bacc.Bacc(
        "TRN2",
        target_bir_lowering=False,
        debug=False,
        enable_asserts=False,
        num_devices=NCORES,
    )
    embT = nc.dram_tensor("embT", [H + 1, PADN], F32, kind="ExternalInput")
    wa = nc.dram_tensor("wa", [H + 1, 2], F32, kind="ExternalInput")
    wb = nc.dram_tensor("wb", [H + 1, 2], F32, kind="ExternalInput")
    psi = nc.dram_tensor("psi", [P, 2], F32, kind="ExternalInput")
    kf = nc.dram_tensor("kf", [P, EV_P], F32, kind="ExternalInput")
    gidx = nc.dram_tensor("gidx", [P, NCH * (CH_I * P // 16)], I16,
                          kind="ExternalInput")
    b0m = nc.dram_tensor("b0m", [P, FREE_I], U8, kind="ExternalInput")
    b1m = nc.dram_tensor("b1m", [P, FREE_I], U8, kind="ExternalInput")
    o_int = nc.dram_tensor("o_int", [P, EV_P], F32, kind="ExternalOutput")
    o_srv = nc.dram_tensor("o_srv", [P, EV_P], F32, kind="ExternalOutput")
    p_shard = nc.dram_tensor("p_shard", [P, 2 * TSH], F32)
    p_full = nc.dram_tensor("p_full", [NCORES * PADN, 2], F32)
    fat_tab = nc.dram_tensor("fat_tab", [NFAT, 64], F32)

    with tile.TileContext(nc) as tc, ExitStack() as ctx:
        const = ctx.enter_context(tc.tile_pool(name="const", bufs=1))
        psum = ctx.enter_context(tc.tile_pool(name="psum", bufs=1, space="PSUM"))
        work = ctx.enter_context(tc.tile_pool(name="work", bufs=3))

        dbg = None
        if debug:
            d_pfull = nc.dram_tensor(
                "d_pfull", [P, NCORES * PADN * 2 // P], F32, kind="ExternalOutput"
            )
            d_gat = nc.dram_tensor("d_gat", [P, CH_I * 2], F32, kind="ExternalOutput")
            dbg = (d_pfull, d_gat)

        def _body():
            _build_body(nc, tc, const, psum, work, embT, wa, wb, psi, kf,
                        gidx, b0m, b1m, o_int, o_srv, p_shard, p_full,
                        fat_tab, dbg)

        for _ in range(reps):
            _body()

    nc.compile()
    return nc


def _build_body(nc, tc, const, psum, work, embT, wa, wb, psi, kf,
                gidx, b0m, b1m, o_int, o_srv, p_shard, p_full, fat_tab,
                dbg=None):
    if True:
        # ---- projection table: P' = [emb | 1] @ [Wc ; b/2] ----
        sb_emb = const.tile([H + 1, PADN], F32)
        nc.sync.dma_start(sb_emb[:], embT[:, :])
        sb_wa = const.tile([H + 1, 2], F32)
        nc.sync.dma_start(sb_wa[:], wa[:, :])
        sb_wb = const.tile([H + 1, 2], F32)
        nc.sync.dma_start(sb_wb[:], wb[:, :])
        sb_wc = const.tile([H + 1, 2], F32)
        nc.vector.tensor_add(sb_wc[:], sb_wa[:], sb_wb[:])
        nc.vector.tensor_scalar(
            out=sb_wc[:], in0=sb_wc[:], scalar1=0.5, scalar2=None,
            op0=mybir.AluOpType.mult,
        )

        ps = psum.tile([P, 2 * TSH], F32)
        for t in range(TSH):
            nc.tensor.matmul(
                out=ps[:, 2 * t:2 * t + 2],
                lhsT=sb_emb[:, t * P:(t + 1) * P],
                rhs=sb_wc[:],
                start=True,
                stop=True,
            )
        sb_p = const.tile([P, 2 * TSH], F32)
        nc.vector.tensor_copy(sb_p[:], ps[:])
        nc.sync.dma_start(p_shard[:, :], sb_p[:])
        nc.gpsimd.collective_compute(
            "AllGather",
            mybir.AluOpType.bypass,
            replica_groups=[list(range(NCORES))],
            ins=[p_shard.ap().opt()],
            outs=[p_full.ap().opt()],
        )

        # spread thin [4f+s, k] rows into 256B-stride fat rows for dma_gather
        nc.sync.dma_start(
            fat_tab.ap()[:, 0:FATE],
            p_full.ap().rearrange("(f s) k -> f (s k)", s=PACK),
        )

        if dbg is not None:
            d_pfull, d_gat = dbg
            fw = NCORES * PADN * 2 // P
            sb_dbg = const.tile([P, fw], F32)
            nc.sync.dma_start(
                sb_dbg[:], p_full.ap().rearrange("(p f) k -> p (f k)", p=P)
            )
            nc.sync.dma_start(d_pfull[:, :], sb_dbg[:])

        # ---- small constants ----
        IW = CH_I * P // 16     # wrapped idx columns per chunk (672)
        sb_gidx = const.tile([P, NCH * IW], I16)
        nc.sync.dma_start(sb_gidx[:], gidx[:, :])
        sb_b0 = const.tile([P, FREE_I], U8)
        nc.sync.dma_start(sb_b0[:], b0m[:, :])
        sb_b1 = const.tile([P, FREE_I], U8)
        nc.sync.dma_start(sb_b1[:], b1m[:, :])
        sb_psi = const.tile([P, 2], F32)
        nc.sync.dma_start(sb_psi[:], psi[:, :])
        sb_kf = const.tile([P, EV_P], F32)
        nc.sync.dma_start(sb_kf[:], kf[:, :])
        sb_psir = const.tile([P, 2], F32)
        nc.vector.reciprocal(sb_psir[:], sb_psi[:])
        sb_psis = const.tile([P, 2], F32)
        nc.vector.tensor_scalar(
            out=sb_psis[:], in0=sb_psi[:], scalar1=1.0 / S, scalar2=None,
            op0=mybir.AluOpType.mult,
        )
        sb_k1h = const.tile([P, 2 * EV_P], F32)
        k1h_r = sb_k1h[:].rearrange("p (e k) -> p e k", k=2)
        nc.vector.tensor_scalar(
            out=k1h_r[:, :, 0], in0=sb_kf[:], scalar1=0.0, scalar2=None,
            op0=mybir.AluOpType.is_equal,
        )
        nc.vector.tensor_scalar(
            out=k1h_r[:, :, 1], in0=sb_kf[:], scalar1=1.0, scalar2=None,
            op0=mybir.AluOpType.is_equal,
        )
        sb_int = const.tile([P, EV_P], F32)
        sb_srv = const.tile([P, EV_P], F32)

        # ---- main loop: gather fat rows, select lane, softplus, reduce ----
        # Q7 idx scratch caps num_idxs at ~1K; split each chunk's 10752
        # lookups into 12 sub-gathers of 896 (7 slots x 128 partitions).
        global _SEMCNT
        for ch in range(NCH):
            fat = work.tile([P, CH_I * FATE], F32, tag="fat")
            fat_r = fat[:].rearrange("p (m e) -> p m e", e=FATE)
            sem = nc.alloc_semaphore(f"gsem{_SEMCNT}")
            _SEMCNT += 1
            for j in range(SUB):
                _emit_dma_gather(
                    nc,
                    out_ap=fat_r[:, SLOT_SUB * j:SLOT_SUB * (j + 1), :],
                    in_ap=fat_tab.ap()[:, 0:FATE],
                    idxs_ap=sb_gidx[:, (ch * SUB + j) * IWS:(ch * SUB + j + 1) * IWS],
                    num_idxs=NI_SUB,
                    elem_size=FATE,
                    elem_step=64,
                ).then_inc(sem, 16)
            b1c = sb_b1[:, ch * CH_I:(ch + 1) * CH_I]
            b0c = sb_b0[:, ch * CH_I:(ch + 1) * CH_I]
            # rows padded to 5/3 f32 so the APs stay 3-dim (interp needs
            # select operands with identical dims; contiguous rows would fuse)
            half = work.tile([P, CH_I * 5], F32, tag="half")
            half_r = half[:].rearrange("p (m e) -> p m e", e=5)
            nc.vector.tensor_copy(
                half_r[:, :, 0:4], fat_r[:, :, 0:4]
            )._wait_ge(sem, 16 * SUB)
            nc.vector.copy_predicated(
                half_r[:, :, 0:4],
                b1c.unsqueeze(2).to_broadcast([P, CH_I, 4]),
                fat_r[:, :, 4:8],
            )
            gat = work.tile([P, CH_I * 3], F32, tag="gat")
            gat_m = gat[:].rearrange("p (m x) -> p m x", x=3)
            nc.vector.select(
                gat_m[:, :, 0:2],
                b0c.unsqueeze(2).to_broadcast([P, CH_I, 2]),
                half_r[:, :, 2:4],
                half_r[:, :, 0:2],
            )
            if dbg is not None and ch == 0:
                nc.sync.dma_start(
                    dbg[1][:, :].rearrange("p (m k) -> p m k", k=2),
                    gat_m[:, :, 0:2],
                )
            gat_e = gat[:].rearrange("p (e m x) -> p e m x", e=CH_EV, x=3)

            g = work.tile([P, CH_EV * EW], F32, tag="g")
            g_r = g[:].rearrange("p (e x) -> p e x", e=CH_EV)
            u2 = gat_e[:, :, 0:1, 0:2].squeeze(2)
            v2 = gat_e[:, :, 1:2, 0:2].squeeze(2)
            # lambda(u, v): P'[u] + P'[v]
            nc.vector.tensor_add(g_r[:, :, 0:2], u2, v2)
            # lambda(u, v_other_s)
            vo = gat_e[:, :, 2:2 + S, 0:2]
            u_b = u2.unsqueeze(2).to_broadcast([P, CH_EV, S, 2])
            nc.vector.tensor_add(
                g_r[:, :, 2:2 + 2 * S].rearrange("p e (s k) -> p e s k", k=2),
                vo, u_b,
            )
            # lambda(v, u_other_s)
            uo = gat_e[:, :, 2 + S:2 + 2 * S, 0:2]
            v_b = v2.unsqueeze(2).to_broadcast([P, CH_EV, S, 2])
            nc.vector.tensor_add(
                g_r[:, :, 2 + 2 * S:2 + 4 * S].rearrange("p e (s k) -> p e s k", k=2),
                uo, v_b,
            )
            # r = g / psi_k
            r = work.tile([P, CH_EV * EW], F32, tag="r")
            g_k = g[:].rearrange("p (x k) -> p x k", k=2)
            r_k = r[:].rearrange("p (x k) -> p x k", k=2)
            psir_b = sb_psir[:].unsqueeze(1).to_broadcast([P, CH_EV * EW // 2, 2])
            nc.vector.tensor_mul(r_k, g_k, psir_b)
            # f = softplus(r) = ln(exp(r) + 1); final lambda = psi * f (scaled below)
            ex = work.tile([P, CH_EV * EW], F32, tag="ex")
            nc.scalar.activation(ex[:], r[:], mybir.ActivationFunctionType.Exp)
            f = work.tile([P, CH_EV * EW], F32, tag="f")
            nc.scalar.activation(
                f[:], ex[:], mybir.ActivationFunctionType.Ln, bias=1.0
            )
            f_r = f[:].rearrange("p (e x) -> p e x", e=CH_EV)

            # survival: psi_k/S * sum over the 40 sampled lambdas
            pairs = f_r[:, :, 2:2 + 4 * S].rearrange(
                "p e (s k) -> p e s k", k=2
            ).transpose([0, 1, 3, 2])
            sv = work.tile([P, CH_EV * 2], F32, tag="sv")
            sv_r = sv[:].rearrange("p (e k) -> p e k", k=2)
            nc.vector.tensor_reduce(
                sv_r, pairs, axis=mybir.AxisListType.X, op=mybir.AluOpType.add
            )
            sv2 = work.tile([P, CH_EV * 2], F32, tag="sv2")
            sv2_r = sv2[:].rearrange("p (e k) -> p e k", k=2)
            psis_b = sb_psis[:].unsqueeze(1).to_broadcast([P, CH_EV, 2])
            nc.vector.tensor_mul(sv2_r, sv_r, psis_b)
            nc.vector.tensor_add(
                sb_srv[:, ch * CH_EV:(ch + 1) * CH_EV],
                sv2_r[:, :, 0], sv2_r[:, :, 1],
            )

            # intensity: psi_k * f at (u, v), k selected by the event type
            tmp = work.tile([P, CH_EV * 2], F32, tag="tmp")
            tmp_r = tmp[:].rearrange("p (e k) -> p e k", k=2)
            k1h_c = sb_k1h[:].rearrange("p (e k) -> p e k", k=2)[
                :, ch * CH_EV:(ch + 1) * CH_EV, :
            ]
            nc.vector.tensor_mul(tmp_r, f_r[:, :, 0:2], k1h_c)
            tmp2 = work.tile([P, CH_EV * 2], F32, tag="tmp2")
            tmp2_r = tmp2[:].rearrange("p (e k) -> p e k", k=2)
            psi_b = sb_psi[:].unsqueeze(1).to_broadcast([P, CH_EV, 2])
            nc.vector.tensor_mul(tmp2_r, tmp_r, psi_b)
            nc.vector.tensor_add(
                sb_int[:, ch * CH_EV:(ch + 1) * CH_EV],
                tmp2_r[:, :, 0], tmp2_r[:, :, 1],
            )

        nc.sync.dma_start(o_int[:, :], sb_int[:])
        nc.sync.dma_start(o_srv[:, :], sb_srv[:])


def _table_row(n):
    """DRAM row of node n in the allgathered, partition-major P table."""
    c = n // RSH
    rr = n % RSH
    return c * PADN + (rr % P) * TSH + (rr // P)


def make_in_maps(u, v, k, u_others, v_others, embeddings, W_omega, b_omega, psi):
    u = np.asarray(u)
    v = np.asarray(v)
    k = np.asarray(k)
    u_others = np.asarray(u_others)
    v_others = np.asarray(v_others)
    embeddings = np.asarray(embeddings, dtype=np.float32)
    W_omega = np.asarray(W_omega, dtype=np.float32)
    b_omega = np.asarray(b_omega, dtype=np.float32)
    psi = np.asarray(psi, dtype=np.float32)

    wa = np.zeros((H + 1, 2), np.float32)
    wa[:H] = W_omega[:, :H].T
    wa[H] = b_omega
    wb = np.zeros((H + 1, 2), np.float32)
    wb[:H] = W_omega[:, H:].T
    psi_rep = np.ascontiguousarray(np.broadcast_to(psi[None, :], (P, 2)))

    in_maps = []
    for c in range(NCORES):
        sl = slice(c * BC, (c + 1) * BC)
        embT = np.zeros((H + 1, PADN), np.float32)
        embT[:H, :RSH] = embeddings[c * RSH:(c + 1) * RSH].T
        embT[H] = 1.0
        idx_ev = np.empty((BC, SLOT), np.int64)
        idx_ev[:, 0] = _table_row(u[sl].astype(np.int64))
        idx_ev[:, 1] = _table_row(v[sl].astype(np.int64))
        idx_ev[:, 2:2 + S] = _table_row(v_others[sl].astype(np.int64))
        idx_ev[:, 2 + S:2 + 2 * S] = _table_row(u_others[sl].astype(np.int64))
        idx_pm = idx_ev.reshape(P, FREE_I)          # [partition, m] table rows
        fat = idx_pm >> 2
        lane = idx_pm & 3
        # dma_gather wrapped int16 indices: lookup j = m*128 + p lands in
        # out[p, m]; wrapped layout puts j at partition j%16, col j//16,
        # replicated across the 8 gpsimd cores.
        gidx_chunks = []
        for ch in range(NCH):
            for j in range(SUB):
                a = ch * CH_I + SLOT_SUB * j
                L = fat[:, a:a + SLOT_SUB].T.reshape(-1)   # 896, m-major
                w16 = L.reshape(-1, 16).T                  # [16, 56]
                gidx_chunks.append(np.tile(w16, (8, 1)))
        in_maps.append(
            dict(
                embT=embT,
                wa=wa,
                wb=wb,
                psi=psi_rep,
                kf=np.ascontiguousarray(
                    k[sl].astype(np.float32).reshape(P, EV_P)
                ),
                gidx=np.ascontiguousarray(
                    np.concatenate(gidx_chunks, axis=1).astype(np.int16)
                ),
                b0m=np.ascontiguousarray((lane & 1).astype(np.uint8)),
                b1m=np.ascontiguousarray((lane >> 1).astype(np.uint8)),
            )
        )
    return in_maps


def kernel(u, v, t, k, u_others, v_others, embeddings, W_omega, b_omega, psi):
    global _PROG, last_results
    if _PROG is None:
        _PROG = build_program()
    in_maps = make_in_maps(
        u, v, k, u_others, v_others, embeddings, W_omega, b_omega, psi
    )
    res = run_bass_kernel_spmd(_PROG, in_maps, core_ids=list(range(NCORES)))
    last_results = res
    out_int = np.concatenate(
        [res.results[c]["o_int"].reshape(BC) for c in range(NCORES)]
    ).astype(np.float32)
    out_srv = np.concatenate(
        [res.results[c]["o_srv"].reshape(BC) for c in range(NCORES)]
    ).astype(np.float32)[:, None]
    return out_int, out_srv
